# revision 2
# baseline (speedup 1.0000x reference)
"""Trainium2 Bass kernel for the isotropic-gaussian differentiable renderer.

Math: for pixel p=(x,y) and gaussian g:
    w[g,p] = op_g * exp(-0.5*((x-ax_g)^2+(y-ay_g)^2)/var_g)
    img[p,c] = (sum_g w[g,p]*col_gc) / (sum_g w[g,p] + n_chunks*EPS)

The isotropic RBF is separable: w = op * exp(sx) * exp(sy) with
sx = s*(x-ax)^2, sy = s*(y-ay)^2, s = -0.5/var.  That turns the
268M-element exp into 2*N*128 exps plus matmuls:

  per 128-gaussian chunk:
    PE (fp32): arg[g, 0:128]=sx(g,x), arg[g,128:256]=sy(g,y) via a K=6
               matmul against fixed rows [u^2,u,1|v^2,v,1] (centered coords;
               fp32 needed: the expansion cancels catastrophically)
    ACT      : expxy = exp(arg)  (PSUM->SBUF, batched over chunks)
    DVE      : A[g, c*128+y] = opc[g,c]*expy[g,y]   (4 tensor_scalar ops)
    PE       : acc[x, c*128+y] += expx^T @ A        (accumulated in PSUM)

Sharding: gaussians split 2048/core across 8 cores; every core accumulates
the full 128x128 image; host sums the 8 partials, divides num/den and
reshapes to the reference's [4,3,64,64] tile layout.
"""
import numpy as np

import concourse.bacc as bacc
import concourse.tile as tile
from concourse import mybir
from concourse.bass_utils import run_bass_kernel_spmd

# Problem constants (hardcoded per harness contract)
N_GAUSS = 16384
H = 128
W = 128
FX = 128.0
FY = 128.0
CX = 64.0
CY = 64.0
EPS = 1e-8
N_CORES = 8
G_PER_CORE = N_GAUSS // N_CORES      # 2048
CHUNK = 128                          # gaussians per matmul chunk
N_CHUNKS = G_PER_CORE // CHUNK       # 16
ARG_W = 256                          # per-chunk arg width: 128 x | 128 y
GROUP = 4                            # chunks per exp batch
N_GROUPS = N_CHUNKS // GROUP         # 4
OUT_W = 512                          # (c,y) free width of the accumulator

F32 = mybir.dt.float32
MM_DT = mybir.dt.float32             # main-accumulation matmul dtype


def build_program():
    """One SPMD Bass program; every core runs it on its gaussian slice."""
    nc = bacc.Bacc("TRN2", target_bir_lowering=False, debug=False,
                   num_devices=N_CORES)
    # [6, 2048]: rows are the K-dim of the arg matmul, cols gaussians.
    coef = nc.dram_tensor("coef", [6, G_PER_CORE], F32, kind="ExternalInput")
    # [6, 256]: fixed moving operand rows [u^2,u,1,0,0,0 | 0,0,0,v^2,v,1]
    rhsxy = nc.dram_tensor("rhsxy", [6, ARG_W], F32, kind="ExternalInput")
    # [128, 64]: opc[p, chunk*4+c] = (op*[r,g,b,1])[chunk*128+p, c]
    opc = nc.dram_tensor("opc", [128, N_CHUNKS * 4], F32, kind="ExternalInput")
    # partial accumulator: [x, c*128+y]
    out = nc.dram_tensor("out", [128, OUT_W], F32, kind="ExternalOutput")

    with tile.TileContext(nc) as tc:
        with tc.tile_pool(name="ins", bufs=1) as ins_pool, \
             tc.tile_pool(name="expp", bufs=1) as exp_pool, \
             tc.tile_pool(name="apool", bufs=3) as a_pool, \
             tc.tile_pool(name="args", bufs=2, space="PSUM") as arg_pool, \
             tc.tile_pool(name="acc", bufs=1, space="PSUM") as acc_pool, \
             tc.tile_pool(name="outp", bufs=1) as out_pool:

            coef_t = ins_pool.tile([6, G_PER_CORE], F32)
            rhs_t = ins_pool.tile([6, ARG_W], F32)
            opc_t = ins_pool.tile([128, N_CHUNKS * 4], F32)
            nc.sync.dma_start(out=coef_t, in_=coef[:, :])
            nc.sync.dma_start(out=rhs_t, in_=rhsxy[:, :])
            nc.sync.dma_start(out=opc_t, in_=opc[:, :])

            # exp(arg) results for all chunks: [g_part, chunk*256 + (x|y)]
            expxy = exp_pool.tile([128, N_CHUNKS * ARG_W], MM_DT)
            acc = acc_pool.tile([128, OUT_W], F32)

            for grp in range(N_GROUPS):
                args = arg_pool.tile([128, GROUP * ARG_W], F32, tag="args")
                for k in range(GROUP):
                    chunk = grp * GROUP + k
                    nc.tensor.matmul(
                        args[:, k * ARG_W:(k + 1) * ARG_W],
                        coef_t[:, chunk * CHUNK:(chunk + 1) * CHUNK],
                        rhs_t[:, :],
                        start=True, stop=True,
                    )
                nc.scalar.activation(
                    out=expxy[:, grp * GROUP * ARG_W:(grp + 1) * GROUP * ARG_W],
                    in_=args[:, :],
                    func=mybir.ActivationFunctionType.Exp,
                )

            for chunk in range(N_CHUNKS):
                ex0 = chunk * ARG_W
                a_t = a_pool.tile([128, OUT_W], MM_DT, tag="a")
                for c in range(4):
                    nc.vector.tensor_scalar_mul(
                        out=a_t[:, c * 128:(c + 1) * 128],
                        in0=expxy[:, ex0 + 128:ex0 + 256],
                        scalar1=opc_t[:, chunk * 4 + c:chunk * 4 + c + 1],
                    )
                nc.tensor.matmul(
                    acc[:, :],
                    expxy[:, ex0:ex0 + 128],
                    a_t[:, :],
                    start=(chunk == 0), stop=(chunk == N_CHUNKS - 1),
                )

            out_t = out_pool.tile([128, OUT_W], F32)
            nc.vector.tensor_copy(out_t, acc)
            nc.sync.dma_start(out=out[:, :], in_=out_t)

    nc.compile()
    return nc


_PROGRAM = None


def _get_program():
    global _PROGRAM
    if _PROGRAM is None:
        _PROGRAM = build_program()
    return _PROGRAM


def _quat2mat(q):
    q = q / np.linalg.norm(q)
    w, x, y, z = q
    return np.array([
        [1 - 2 * (y * y + z * z), 2 * (x * y - z * w), 2 * (x * z + y * w)],
        [2 * (x * y + z * w), 1 - 2 * (x * x + z * z), 2 * (y * z - x * w)],
        [2 * (x * z - y * w), 2 * (y * z + x * w), 1 - 2 * (x * x + y * y)],
    ])


def kernel(positions, colors, opacities, scales, qvec, tvec, tile_hw,
           chunk_gauss, _trace=False):
    positions = np.asarray(positions, dtype=np.float32)
    colors = np.asarray(colors, dtype=np.float32)
    opacities = np.asarray(opacities, dtype=np.float32)
    scales = np.asarray(scales, dtype=np.float32)
    qvec = np.asarray(qvec, dtype=np.float32)
    tvec = np.asarray(tvec, dtype=np.float32)
    tile_hw = int(tile_hw)
    chunk_gauss = int(chunk_gauss)
    n = positions.shape[0]
    assert n == N_GAUSS, f"expected {N_GAUSS} gaussians, got {n}"

    # ---- O(N) per-gaussian prep in float64 (rounds to the same f32 values
    # the reference computes, to well within the exp's own error budget) ----
    R = _quat2mat(qvec.astype(np.float64))
    cam = positions.astype(np.float64) @ R.T + tvec.astype(np.float64)
    ax = cam[:, 0] / cam[:, 2] * FX + CX          # [N] screen x center
    ay = cam[:, 1] / cam[:, 2] * FY + CY          # [N] screen y center
    var = scales[:, 0].astype(np.float64) ** 2
    s = -0.5 / var                                # [N] negative inv 2*var

    # centered coords keep the quadratic-expansion terms small (|u|<=64)
    dx = ax - CX
    dy = ay - CY
    # rows of the K=6 stationary operand, per gaussian:
    #   arg_x = s*u^2 + (-2 s dx)*u + s*dx^2     (u = x - 64)
    #   arg_y = s*v^2 + (-2 s dy)*v + s*dy^2     (v = y - 64)
    coef_full = np.stack([
        s, -2.0 * s * dx, s * dx * dx,
        s, -2.0 * s * dy, s * dy * dy,
    ]).astype(np.float32)                         # [6, N]

    u = np.arange(W, dtype=np.float64) - CX
    v = np.arange(H, dtype=np.float64) - CY
    zeros = np.zeros(128)
    rhsxy = np.stack([
        np.concatenate([u * u, zeros]),
        np.concatenate([u, zeros]),
        np.concatenate([np.ones(128), zeros]),
        np.concatenate([zeros, v * v]),
        np.concatenate([zeros, v]),
        np.concatenate([zeros, np.ones(128)]),
    ]).astype(np.float32)                         # [6, 256]

    op = opacities[:, 0].astype(np.float64)
    opc_full = np.concatenate(
        [colors.astype(np.float64) * op[:, None], op[:, None]], axis=1
    ).astype(np.float32)                          # [N, 4] = op*[r,g,b,1]

    # ---- shard gaussians across the 8 cores ----
    in_maps = []
    for core in range(N_CORES):
        g0 = core * G_PER_CORE
        g1 = g0 + G_PER_CORE
        opc_c = opc_full[g0:g1].reshape(N_CHUNKS, CHUNK, 4)
        opc_c = np.ascontiguousarray(
            opc_c.transpose(1, 0, 2).reshape(CHUNK, N_CHUNKS * 4))
        in_maps.append({
            "coef": np.ascontiguousarray(coef_full[:, g0:g1]),
            "rhsxy": rhsxy,
            "opc": opc_c,
        })

    nc = _get_program()
    res = run_bass_kernel_spmd(nc, in_maps, list(range(N_CORES)),
                               trace=_trace)

    # ---- host reduction: sum per-core partials, divide, reshape ----
    acc = np.zeros((128, 4, 128), dtype=np.float64)   # [x, c, y]
    for core in range(N_CORES):
        acc += res.results[core]["out"].reshape(128, 4, 128)

    num = acc[:, 0:3, :]                          # [x, c, y]
    n_chunks_ref = n // chunk_gauss
    den = acc[:, 3, :] + n_chunks_ref * EPS       # [x, y]
    img = num / den[:, None, :]                   # [x, c, y]
    img = img.transpose(2, 0, 1).reshape(H * W, 3)  # [p=(y,x), c]

    step = tile_hw * tile_hw
    t = (H * W) // step
    out = img.reshape(t, step, 3).transpose(0, 2, 1).reshape(
        t, 3, tile_hw, tile_hw)
    result = out.astype(np.float32)
    if _trace:
        return result, res
    return result
